# revision 8
# baseline (speedup 1.0000x reference)
"""Max-SW loss kernel for Trainium2 (8 NeuronCores, data-parallel over batch).

Algorithm (per batch element, 4 per core):
  State: records (K = x@p sort key fp32, packed bf16 coords c0,c1) for both
  point clouds, kept physically sorted; c2 recovered as (K - c0 p0 - c1 p1)/p2
  (host permutes axes so |p2| is maximal).
  Host pre-sorts by the initial projection. Each Adam iteration on-device:
    1. gradient reductions on the position-paired sorted arrays
       g_p = 2 sum_n d_n (X_s[n]-Y_s[n]),  d = Kx - Ky
    2. Adam update of u (3-vector), new direction p, delta = p_new - p_old
    3. incremental key update K <- K*(1+d2/p2) + c0*(d0-d2 p0/p2) + c1*(...)
    4. re-sort repair: 20 decreasing-gap odd-even compare-exchange stages
       (validated numerically: residual disorder stays bounded, final loss
       rel err ~1e-5 vs exact float64 reference)
  Final: cleanup stages + sum d^2 per batch; host averages 32 batches.

Layout: per core 8 arrays (4 batches x {x,y}) interleaved in fat planes
[128, 8192]; rank r = row*1024 + f, fat column index = f*8 + array.
"""
import numpy as np
import ml_dtypes

import concourse.bacc as bacc
import concourse.bass as bass
import concourse.tile as tile
from concourse import mybir
from concourse.bass_utils import run_bass_kernel_spmd

f32 = mybir.dt.float32
u32 = mybir.dt.uint32
u8 = mybir.dt.uint8
bf16 = mybir.dt.bfloat16
Alu = mybir.AluOpType
Act = mybir.ActivationFunctionType
Axis = mybir.AxisListType

NUM_ITER = 50
NCORES = 8
B_PER_CORE = 4
NARR = 8          # arrays per core = 4 batches * (x, y)
ROWS, FPR = 128, 1024   # rank = row*1024 + f
N = ROWS * FPR
FAT = FPR * NARR  # 8192

# exact float32 constants as used by the jax fp32 reference
LRf = float(np.float32(1e-4))
B1f = float(np.float32(0.9))
B2f = float(np.float32(0.999))
OneMinusB1 = float(np.float32(1.0) - np.float32(0.9))
OneMinusB2 = float(np.float32(1.0) - np.float32(0.999))
EPSf = float(np.float32(1e-8))

# (gap, phase) repair schedule, validated in numpy mirror
GAPS = [(64, 0), (64, 1), (32, 0), (32, 1), (16, 0), (16, 1), (8, 0), (8, 1),
        (4, 0), (4, 1), (4, 0), (2, 0), (2, 1), (2, 0), (1, 0), (1, 1),
        (1, 0), (1, 1), (1, 0), (1, 1)]
BSCHED = {0: 64, 3: 32, 9: 16, 14: 4}   # boundary-exchange after stage idx
CLEANUP = [(8, 0), (4, 0), (2, 0), (2, 1), (1, 0), (1, 1)]
BSCHED_CLEAN = {0: 8, 3: 4}
assert len(GAPS) % 2 == 0 and len(CLEANUP) % 2 == 0


def bcast_inner(ap, n):
    """Append a step-0 inner dim of count n to an AP (broadcast)."""
    return bass.AP(tensor=ap.tensor, offset=ap.offset, ap=list(ap.ap) + [[0, n]])


def build_nc(num_iter=NUM_ITER):
    nc = bacc.Bacc("TRN2", target_bir_lowering=False, debug=False,
                   num_devices=NCORES)
    kin = nc.dram_tensor("kin", [ROWS, FAT], f32, kind="ExternalInput").ap()
    pin = nc.dram_tensor("pin", [ROWS, FAT], u32, kind="ExternalInput").ap()
    scin = nc.dram_tensor("scin", [1, 24], f32, kind="ExternalInput").ap()
    out_d = nc.dram_tensor("out", [1, 16], f32, kind="ExternalOutput").ap()

    with tile.TileContext(nc) as tc:
        with (
            tc.tile_pool(name="planes", bufs=1) as planes,
            tc.tile_pool(name="small", bufs=1) as small,
            tc.tile_pool(name="ps", bufs=1, space="PSUM") as psp,
        ):
            AK = planes.tile([ROWS, FAT], f32, tag="AK")
            BK = planes.tile([ROWS, FAT], f32, tag="BK")
            AP_ = planes.tile([ROWS, FAT], u32, tag="AP")
            BP_ = planes.tile([ROWS, FAT], u32, tag="BP")
            MASK = planes.tile([ROWS, 4096], u8, tag="MASK")
            MASK2 = small.tile([ROWS, 512], u8)

            SCB = small.tile([ROWS, 16], f32)
            ACC = small.tile([ROWS, 16], f32)
            ONES = small.tile([ROWS, 1], f32)
            TU = small.tile([1, 12], f32)
            TM = small.tile([1, 12], f32)
            TV = small.tile([1, 12], f32)
            TP = small.tile([1, 12], f32)
            TPN = small.tile([1, 12], f32)
            TG = small.tile([1, 12], f32)
            TS1 = small.tile([1, 12], f32)
            TS2 = small.tile([1, 12], f32)
            TD4 = small.tile([1, 4], f32)
            TN4 = small.tile([1, 4], f32)
            TBC = small.tile([1, 2], f32)
            TBCI = small.tile([1, 2], f32)
            TRC4 = small.tile([1, 4], f32)
            TRC2 = small.tile([1, 2], f32)
            TRC12 = small.tile([1, 12], f32)
            TR = small.tile([1, 16], f32)
            SCOUT = small.tile([1, 16], f32)
            JUNK = small.tile([ROWS, 16], f32)
            ONESR = small.tile([1, ROWS], f32)
            SHK = small.tile([ROWS, 512], f32)
            SHP = small.tile([ROWS, 512], f32)
            SH2K = small.tile([ROWS, 512], f32)
            SH2P = small.tile([ROWS, 512], f32)
            PSUMT = psp.tile([1, 16], f32)
            PSB = psp.tile([ROWS, 16], f32)

            # ---------- prologue ----------
            nc.sync.dma_start(out=AK[:], in_=kin)
            nc.sync.dma_start(out=AP_[:], in_=pin)
            nc.sync.dma_start(out=TU[:], in_=scin[0:1, 0:12])
            nc.sync.dma_start(out=TP[:], in_=scin[0:1, 12:24])
            nc.vector.memset(TM[:], 0.0)
            nc.vector.memset(TV[:], 0.0)
            nc.vector.memset(TBC[:], 1.0)
            nc.vector.memset(ONES[:], 1.0)
            nc.vector.memset(ONESR[:], 1.0)
            nc.vector.memset(BK[:], 0.0)
            nc.vector.memset(BP_[:], 0)
            nc.vector.memset(MASK[:], 0)
            nc.vector.memset(MASK2[:], 0)
            nc.vector.memset(ACC[:], 0.0)
            nc.vector.memset(SCB[:], 0.0)
            nc.vector.memset(JUNK[:], 0.0)
            nc.vector.memset(SHK[:], 0.0)
            nc.vector.memset(SHP[:], 0.0)
            nc.vector.memset(SH2K[:], 0.0)
            nc.vector.memset(SH2P[:], 0.0)

            # helper views -------------------------------------------------
            def kview(t):
                return t[:].rearrange("p (f a) -> p f a", a=NARR)

            def cview(t, h):
                # bf16 coord view: h=1 -> c0 (high half), h=0 -> c1 (low)
                v = t[:].bitcast(bf16).rearrange(
                    "p (f a h) -> p f a h", a=NARR, h=2)
                return v[:, :, :, h]

            def reductions(kt, pt, final=False):
                """d, and per-batch accumulators into ACC."""
                kv = kview(kt)
                bkv = kview(BK if kt is AK else AK)
                c0 = cview(pt, 1)
                c1 = cview(pt, 0)
                bscr = (BP_ if pt is AP_ else AP_)[:].bitcast(f32).rearrange(
                    "p (f a) -> p f a", a=NARR)
                for b in range(B_PER_CORE):
                    ax, ay = 2 * b, 2 * b + 1
                    D = bkv[:, :, ax]
                    # D = Kx - Ky
                    nc.vector.tensor_tensor(D, kv[:, :, ax], kv[:, :, ay],
                                            Alu.subtract)
                    # sum d^2 (ACT engine, fused square+accum)
                    nc.scalar.activation(bkv[:, :, ay], D, Act.Square,
                                         accum_out=ACC[:, 4 * b:4 * b + 1])
                    if not final:
                        U0 = bscr[:, :, ax]
                        U1 = bscr[:, :, ay]
                        nc.vector.tensor_tensor(U0, c0[:, :, ax], c0[:, :, ay],
                                                Alu.subtract)
                        nc.vector.scalar_tensor_tensor(
                            U0, U0, 1.0, D, Alu.mult, Alu.mult,
                            accum_out=ACC[:, 4 * b + 1:4 * b + 2])
                        nc.vector.tensor_tensor(U1, c1[:, :, ax], c1[:, :, ay],
                                                Alu.subtract)
                        nc.vector.scalar_tensor_tensor(
                            U1, U1, 1.0, D, Alu.mult, Alu.mult,
                            accum_out=ACC[:, 4 * b + 2:4 * b + 3])
                # collect across partitions: PSUM[1,16] = ones^T @ ACC
                nc.tensor.matmul(PSUMT[0:1, :], ONES[:, 0:1], ACC[:, :],
                                 start=True, stop=True)
                nc.scalar.copy(TR[:], PSUMT[0:1, :])

            def adam_and_scalars():
                """TR -> gradient -> adam -> SCOUT (s,alpha,beta per batch) + SCB."""
                r = TR[:].rearrange("o (b q) -> o b q", q=4)
                sd2, su0, su1 = r[:, :, 0], r[:, :, 1], r[:, :, 2]
                tp3 = TP[:].rearrange("o (b c) -> o b c", c=3)
                p0o, p1o, p2o = tp3[:, :, 0], tp3[:, :, 1], tp3[:, :, 2]
                ts4 = TS1[:].rearrange("o (b c) -> o b c", c=3)
                # gp2*0.5 = (sd2 - p0*su0 - p1*su1)/p2
                nc.vector.tensor_tensor(ts4[:, :, 0], su0, p0o, Alu.mult)
                nc.vector.tensor_tensor(ts4[:, :, 1], su1, p1o, Alu.mult)
                nc.vector.tensor_tensor(ts4[:, :, 2], sd2, ts4[:, :, 0],
                                        Alu.subtract)
                nc.vector.tensor_tensor(ts4[:, :, 2], ts4[:, :, 2],
                                        ts4[:, :, 1], Alu.subtract)
                nc.vector.reciprocal(TRC4[:], p2o)
                nc.vector.tensor_tensor(ts4[:, :, 2], ts4[:, :, 2], TRC4[:],
                                        Alu.mult)
                tg3 = TG[:].rearrange("o (b c) -> o b c", c=3)
                nc.vector.tensor_scalar_mul(tg3[:, :, 0], su0, 2.0)
                nc.vector.tensor_scalar_mul(tg3[:, :, 1], su1, 2.0)
                nc.vector.tensor_scalar_mul(tg3[:, :, 2], ts4[:, :, 2], 2.0)
                # dot = sum gp*p per batch; gp_tan = gp - dot*p
                nc.vector.tensor_tensor(TS2[:], TG[:], TP[:], Alu.mult)
                nc.vector.tensor_reduce(
                    TD4[:], TS2[:].rearrange("o (b c) -> o b c", c=3),
                    Axis.X, Alu.add)
                d4b = bcast_inner(TD4[0:1, :], 3)
                nc.vector.tensor_tensor(TS2[:], TP[:], d4b, Alu.mult)
                nc.vector.tensor_tensor(TG[:], TG[:], TS2[:], Alu.subtract)
                # nrm = |u|; gu = -gp_tan/(nrm*32)
                nc.vector.tensor_tensor(TS2[:], TU[:], TU[:], Alu.mult)
                nc.vector.tensor_reduce(
                    TN4[:], TS2[:].rearrange("o (b c) -> o b c", c=3),
                    Axis.X, Alu.add)
                nc.scalar.activation(TN4[:], TN4[:], Act.Sqrt)
                nc.vector.reciprocal(TRC4[:], TN4[:])
                nc.vector.tensor_tensor(TG[:], TG[:], bcast_inner(TRC4[0:1, :], 3),
                                        Alu.mult)
                nc.vector.tensor_scalar_mul(TG[:], TG[:], -1.0 / 32.0)
                # adam moments
                nc.vector.tensor_scalar_mul(TS1[:], TG[:], OneMinusB1)
                nc.vector.scalar_tensor_tensor(TM[:], TM[:], B1f, TS1[:],
                                               Alu.mult, Alu.add)
                nc.vector.tensor_tensor(TS2[:], TG[:], TG[:], Alu.mult)
                nc.vector.tensor_scalar_mul(TS2[:], TS2[:], OneMinusB2)
                nc.vector.scalar_tensor_tensor(TV[:], TV[:], B2f, TS2[:],
                                               Alu.mult, Alu.add)
                # bias correction factors
                nc.vector.tensor_scalar_mul(TBC[0:1, 0:1], TBC[0:1, 0:1], B1f)
                nc.vector.tensor_scalar_mul(TBC[0:1, 1:2], TBC[0:1, 1:2], B2f)
                nc.vector.tensor_scalar(TBCI[:], TBC[:], -1.0, 1.0,
                                        Alu.mult, Alu.add)
                nc.vector.reciprocal(TRC2[:], TBCI[:])
                nc.vector.tensor_tensor(TS1[:], TM[:],
                                        bcast_inner(TRC2[0:1, 0:1], 12),
                                        Alu.mult)
                nc.vector.tensor_tensor(TS2[:], TV[:],
                                        bcast_inner(TRC2[0:1, 1:2], 12),
                                        Alu.mult)
                # u -= lr*mhat/(sqrt(vhat)+eps)
                nc.scalar.activation(TS2[:], TS2[:], Act.Sqrt)
                nc.vector.tensor_scalar_add(TS2[:], TS2[:], EPSf)
                nc.vector.tensor_scalar_mul(TS1[:], TS1[:], LRf)
                nc.vector.reciprocal(TRC12[:], TS2[:])
                nc.vector.tensor_tensor(TS1[:], TS1[:], TRC12[:], Alu.mult)
                nc.vector.tensor_tensor(TU[:], TU[:], TS1[:], Alu.subtract)
                # p_new = u/|u|
                nc.vector.tensor_tensor(TS2[:], TU[:], TU[:], Alu.mult)
                nc.vector.tensor_reduce(
                    TN4[:], TS2[:].rearrange("o (b c) -> o b c", c=3),
                    Axis.X, Alu.add)
                nc.scalar.activation(TN4[:], TN4[:], Act.Sqrt)
                nc.vector.reciprocal(TRC4[:], TN4[:])
                nc.vector.tensor_tensor(TPN[:], TU[:],
                                        bcast_inner(TRC4[0:1, :], 3), Alu.mult)
                # delta and per-batch key-update scalars
                nc.vector.tensor_tensor(TS1[:], TPN[:], TP[:], Alu.subtract)
                dl3 = TS1[:].rearrange("o (b c) -> o b c", c=3)
                sc4 = SCOUT[:].rearrange("o (b q) -> o b q", q=4)
                nc.vector.reciprocal(TRC4[:], p2o)
                nc.vector.tensor_tensor(TD4[:], dl3[:, :, 2], TRC4[:], Alu.mult)
                nc.vector.tensor_scalar_add(sc4[:, :, 0], TD4[:], 1.0)
                nc.vector.tensor_tensor(TN4[:], TD4[:], p0o, Alu.mult)
                nc.vector.tensor_tensor(sc4[:, :, 1], dl3[:, :, 0], TN4[:],
                                        Alu.subtract)
                nc.vector.tensor_tensor(TN4[:], TD4[:], p1o, Alu.mult)
                nc.vector.tensor_tensor(sc4[:, :, 2], dl3[:, :, 1], TN4[:],
                                        Alu.subtract)
                nc.vector.tensor_copy(TP[:], TPN[:])
                # broadcast to all partitions via PE outer product
                nc.tensor.matmul(PSB[:, :], ONESR[0:1, :], SCOUT[:, :],
                                 start=True, stop=True)
                nc.scalar.copy(SCB[:], PSB[:, :])

            def key_update():
                kv = kview(AK)
                c0 = cview(AP_, 1)
                c1 = cview(AP_, 0)
                for b in range(B_PER_CORE):
                    ks = kv[:, :, 2 * b:2 * b + 2]
                    nc.vector.tensor_scalar_mul(ks, ks, SCB[:, 4 * b:4 * b + 1])
                    nc.vector.scalar_tensor_tensor(
                        ks, c0[:, :, 2 * b:2 * b + 2],
                        SCB[:, 4 * b + 1:4 * b + 2], ks, Alu.mult, Alu.add)
                    nc.vector.scalar_tensor_tensor(
                        ks, c1[:, :, 2 * b:2 * b + 2],
                        SCB[:, 4 * b + 2:4 * b + 3], ks, Alu.mult, Alu.add)

            def stage(g, ph, srcK, dstK, srcP, dstP):
                B = FPR // (2 * g)
                for t, s, d in ((0, srcK, dstK), (1, srcP, dstP)):
                    sap = s[:] if t == 0 else s[:].bitcast(f32)
                    dap = d[:] if t == 0 else d[:].bitcast(f32)
                    sv = sap.rearrange("p (b two j a) -> p b two j a",
                                       two=2, j=g, a=NARR)
                    dv = dap.rearrange("p (b two j a) -> p b two j a",
                                       two=2, j=g, a=NARR)
                    if ph == 0:
                        slo, shi = sv[:, :, 0], sv[:, :, 1]
                        dlo, dhi = dv[:, :, 0], dv[:, :, 1]
                        mv = MASK[:].rearrange("p (b j a) -> p b j a",
                                               j=g, a=NARR)
                    else:
                        slo, shi = sv[:, 0:B - 1, 1], sv[:, 1:B, 0]
                        dlo, dhi = dv[:, 0:B - 1, 1], dv[:, 1:B, 0]
                        mv = MASK[:].rearrange("p (b j a) -> p b j a",
                                               j=g, a=NARR)[:, 0:B - 1]
                    if t == 0:
                        nc.vector.tensor_tensor(mv, slo, shi, Alu.is_gt)
                        nc.vector.tensor_tensor(dlo, slo, shi, Alu.min)
                        nc.vector.tensor_tensor(dhi, slo, shi, Alu.max)
                    else:
                        nc.scalar.copy(dlo, slo)
                        nc.scalar.copy(dhi, shi)
                        nc.vector.copy_predicated(dlo, mv, shi)
                        nc.vector.copy_predicated(dhi, mv, slo)
                    if ph == 1:
                        # uncovered row-edge regions: plain copies
                        fv_s = sap.rearrange("p (f a) -> p f a", a=NARR)
                        fv_d = dap.rearrange("p (f a) -> p f a", a=NARR)
                        nc.scalar.copy(fv_d[:, 0:g, :], fv_s[:, 0:g, :])
                        nc.scalar.copy(fv_d[:, FPR - g:FPR, :],
                                       fv_s[:, FPR - g:FPR, :])

            def boundary_event(w, curK, curP):
                """merge-exchange row tails (rows 0..126) vs next-row heads,
                in place on the current buffers, via DMA staging."""
                W = w * NARR
                kf = curK[:].rearrange("p (f a) -> p f a", a=NARR)
                pf = curP[:].bitcast(f32).rearrange("p (f a) -> p f a", a=NARR)
                ktail = kf[0:ROWS - 1, FPR - w:FPR, :]
                ptail = pf[0:ROWS - 1, FPR - w:FPR, :]
                khead = kf[1:ROWS, 0:w, :]
                phead = pf[1:ROWS, 0:w, :]
                shk = SHK[0:ROWS - 1, 0:W]
                shp = SHP[0:ROWS - 1, 0:W]
                sh2k = SH2K[0:ROWS - 1, 0:W]
                sh2p = SH2P[0:ROWS - 1, 0:W]
                m2 = MASK2[0:ROWS - 1, 0:W]
                # stage heads of rows 1.. at partitions 0..
                nc.sync.dma_start(out=shk, in_=khead)
                nc.sync.dma_start(out=shp, in_=phead)
                nc.vector.tensor_tensor(m2, ktail, shk, Alu.is_gt)
                # new head values
                nc.vector.tensor_tensor(sh2k, ktail, shk, Alu.max)
                nc.scalar.copy(sh2p, shp)
                nc.vector.copy_predicated(sh2p, m2, ptail)
                # in-place tail update
                nc.vector.tensor_tensor(ktail, ktail, shk, Alu.min)
                nc.vector.copy_predicated(ptail, m2, shp)
                # write back heads
                nc.sync.dma_start(out=khead, in_=sh2k)
                nc.sync.dma_start(out=phead, in_=sh2p)

            def repair(gaps, bsched):
                bufs = [(AK, AP_), (BK, BP_)]
                cur = 0
                for i, (g, ph) in enumerate(gaps):
                    (sK, sP), (dK, dP) = bufs[cur], bufs[1 - cur]
                    stage(g, ph, sK, dK, sP, dP)
                    cur = 1 - cur
                    if i in bsched:
                        boundary_event(bsched[i], bufs[cur][0], bufs[cur][1])
                assert cur == 0

            # ---------- main loop ----------
            def body(iv):
                reductions(AK, AP_)
                adam_and_scalars()
                key_update()
                repair(GAPS, BSCHED)

            with tc.For_i(0, num_iter, 1) as iv:
                body(iv)

            # ---------- epilogue ----------
            repair(CLEANUP, BSCHED_CLEAN)
            reductions(AK, AP_, final=True)
            nc.sync.dma_start(out=out_d, in_=TR[:])

    nc.compile()
    return nc


_NC_CACHE = {}


def _get_nc(num_iter=NUM_ITER):
    if num_iter not in _NC_CACHE:
        _NC_CACHE[num_iter] = build_nc(num_iter)
    return _NC_CACHE[num_iter]


def _prep_core(xc, yc, pc, num_iter):
    """Host-side prep for one core: returns the in_map."""
    KIN = np.empty((ROWS, FAT), np.float32)
    PIN = np.empty((ROWS, FAT), np.uint32)
    SCIN = np.empty((1, 24), np.float32)
    for b in range(B_PER_CORE):
        u0 = pc[b, 0].astype(np.float32)
        nrm = np.sqrt((u0.astype(np.float32) ** 2).sum(dtype=np.float32))
        p0 = (u0 / nrm).astype(np.float32)
        perm = np.argsort(np.abs(p0), kind="stable")
        xb = xc[b][:, perm]
        yb = yc[b][:, perm]
        p0p = p0[perm]
        u0p = u0[perm]
        SCIN[0, 3 * b:3 * b + 3] = u0p
        SCIN[0, 12 + 3 * b:12 + 3 * b + 3] = p0p
        for cloud, arr in ((0, xb), (1, yb)):
            a = 2 * b + cloud
            proj = (arr @ p0p).astype(np.float32)
            order = np.argsort(proj, kind="stable")
            k = proj[order]
            c0 = arr[order, 0].astype(ml_dtypes.bfloat16)
            c1 = arr[order, 1].astype(ml_dtypes.bfloat16)
            packed = (c0.view(np.uint16).astype(np.uint32) << 16) | \
                c1.view(np.uint16).astype(np.uint32)
            KIN[:, a::NARR] = k.reshape(ROWS, FPR)
            PIN[:, a::NARR] = packed.reshape(ROWS, FPR)
    return {"kin": KIN, "pin": PIN, "scin": SCIN}


def kernel(x, y, proj_init, num_iter=NUM_ITER):
    x = np.asarray(x)
    y = np.asarray(y)
    proj_init = np.asarray(proj_init)
    Btot = x.shape[0]
    assert Btot == NCORES * B_PER_CORE
    nc = _get_nc(num_iter)
    in_maps = []
    for c in range(NCORES):
        sl = slice(c * B_PER_CORE, (c + 1) * B_PER_CORE)
        in_maps.append(_prep_core(x[sl], y[sl], proj_init[sl], num_iter))
    res = run_bass_kernel_spmd(nc, in_maps, core_ids=list(range(NCORES)))
    svals = []
    for c in range(NCORES):
        o = res.results[c]["out"]
        for b in range(B_PER_CORE):
            svals.append(o[0, 4 * b])
    return np.float32(np.mean(np.asarray(svals, np.float64)))


# revision 12
# speedup vs baseline: 86.2128x; 86.2128x over previous
"""Max-SW loss kernel for Trainium2 (8 NeuronCores, data-parallel over batch).

Algorithm (per batch element, 4 per core):
  State: records (K = x@p sort key fp32, packed bf16 coords c0,c1) for both
  point clouds, kept physically sorted; c2 recovered as (K - c0 p0 - c1 p1)/p2
  (host permutes axes so |p2| is maximal).
  Host pre-sorts by the initial projection. Each Adam iteration on-device:
    1. gradient reductions on the position-paired sorted arrays
       g_p = 2 sum_n d_n (X_s[n]-Y_s[n]),  d = Kx - Ky
    2. Adam update of u (3-vector), new direction p, delta = p_new - p_old
    3. incremental key update K <- K*(1+d2/p2) + c0*(d0-d2 p0/p2) + c1*(...)
    4. re-sort repair: 20 decreasing-gap odd-even compare-exchange stages
       (validated numerically: residual disorder stays bounded, final loss
       rel err ~1e-5 vs exact float64 reference)
  Final: cleanup stages + sum d^2 per batch; host averages 32 batches.

Layout: per core 8 arrays (4 batches x {x,y}) interleaved in fat planes
[128, 8192]; rank r = row*1024 + f, fat column index = f*8 + array.
"""
import numpy as np
import ml_dtypes

import concourse.bacc as bacc
import concourse.bass as bass
import concourse.tile as tile
from concourse import mybir
from concourse.bass_utils import run_bass_kernel_spmd

f32 = mybir.dt.float32
u32 = mybir.dt.uint32
u8 = mybir.dt.uint8
bf16 = mybir.dt.bfloat16
Alu = mybir.AluOpType
Act = mybir.ActivationFunctionType
Axis = mybir.AxisListType

NUM_ITER = 50
NCORES = 8
B_PER_CORE = 4
NARR = 8          # arrays per core = 4 batches * (x, y)
ROWS, FPR = 128, 1024   # rank = row*1024 + f
N = ROWS * FPR
FAT = FPR * NARR  # 8192

# exact float32 constants as used by the jax fp32 reference
LRf = float(np.float32(1e-4))
B1f = float(np.float32(0.9))
B2f = float(np.float32(0.999))
OneMinusB1 = float(np.float32(1.0) - np.float32(0.9))
OneMinusB2 = float(np.float32(1.0) - np.float32(0.999))
EPSf = float(np.float32(1e-8))

# (gap, phase) repair schedule, validated in numpy mirror
GAPS = [(64, 0), (64, 1), (32, 0), (32, 1), (16, 0), (16, 1), (8, 0), (8, 1),
        (4, 0), (4, 1), (4, 0), (2, 0), (2, 1), (2, 0), (1, 0), (1, 1),
        (1, 0), (1, 1), (1, 0), (1, 1)]
BSCHED = {1: 64, 7: 32, 13: 8}   # boundary-exchange after stage idx
CLEANUP = [(8, 0), (4, 0), (2, 0), (2, 1), (1, 0), (1, 1)]
BSCHED_CLEAN = {0: 8, 3: 4}
assert len(GAPS) % 2 == 0 and len(CLEANUP) % 2 == 0


def bcast_inner(ap, n):
    """Append a step-0 inner dim of count n to an AP (broadcast)."""
    return bass.AP(tensor=ap.tensor, offset=ap.offset, ap=list(ap.ap) + [[0, n]])


def build_nc(num_iter=NUM_ITER):
    nc = bacc.Bacc("TRN2", target_bir_lowering=False, debug=False,
                   num_devices=NCORES)
    kin = nc.dram_tensor("kin", [ROWS, FAT], f32, kind="ExternalInput").ap()
    pin = nc.dram_tensor("pin", [ROWS, FAT], u32, kind="ExternalInput").ap()
    scin = nc.dram_tensor("scin", [1, 24], f32, kind="ExternalInput").ap()
    out_d = nc.dram_tensor("out", [1, 16], f32, kind="ExternalOutput").ap()

    with tile.TileContext(nc) as tc:
        with (
            tc.tile_pool(name="planes", bufs=1) as planes,
            tc.tile_pool(name="small", bufs=1) as small,
            tc.tile_pool(name="ps", bufs=1, space="PSUM") as psp,
        ):
            AK = planes.tile([ROWS, FAT], f32, tag="AK")
            BK = planes.tile([ROWS, FAT], f32, tag="BK")
            AP_ = planes.tile([ROWS, FAT], u32, tag="AP")
            BP_ = planes.tile([ROWS, FAT], u32, tag="BP")
            MASK = planes.tile([ROWS, 4096], u8, tag="MASK")
            MASK2 = small.tile([ROWS, 512], u8)

            SCB = small.tile([ROWS, 16], f32)
            ACC = small.tile([ROWS, 16], f32)
            ONES = small.tile([ROWS, 1], f32)
            TU = small.tile([1, 12], f32)
            TM = small.tile([1, 12], f32)
            TV = small.tile([1, 12], f32)
            TP = small.tile([1, 12], f32)
            TPN = small.tile([1, 12], f32)
            TG = small.tile([1, 12], f32)
            TS1 = small.tile([1, 12], f32)
            TS2 = small.tile([1, 12], f32)
            TD4 = small.tile([1, 4], f32)
            TN4 = small.tile([1, 4], f32)
            TBC = small.tile([1, 2], f32)
            TBCI = small.tile([1, 2], f32)
            TRC4 = small.tile([1, 4], f32)
            TRC2 = small.tile([1, 2], f32)
            TRC12 = small.tile([1, 12], f32)
            TR = small.tile([1, 16], f32)
            SCOUT = small.tile([1, 16], f32)
            JUNK = small.tile([ROWS, 16], f32)
            ONESR = small.tile([1, ROWS], f32)
            SHK = small.tile([ROWS, 512], f32)
            SHP = small.tile([ROWS, 512], f32)
            SH2K = small.tile([ROWS, 512], f32)
            SH2P = small.tile([ROWS, 512], f32)
            PSUMT = psp.tile([1, 16], f32)
            PSB = psp.tile([ROWS, 16], f32)

            # ---------- prologue ----------
            nc.sync.dma_start(out=AK[:], in_=kin)
            nc.sync.dma_start(out=AP_[:], in_=pin)
            nc.sync.dma_start(out=TU[:], in_=scin[0:1, 0:12])
            nc.sync.dma_start(out=TP[:], in_=scin[0:1, 12:24])
            nc.vector.memset(TM[:], 0.0)
            nc.vector.memset(TV[:], 0.0)
            nc.vector.memset(TBC[:], 1.0)
            nc.vector.memset(ONES[:], 1.0)
            nc.vector.memset(ONESR[:], 1.0)
            nc.vector.memset(BK[:], 0.0)
            nc.vector.memset(BP_[:], 0)
            nc.vector.memset(MASK[:], 0)
            nc.vector.memset(MASK2[:], 0)
            nc.vector.memset(ACC[:], 0.0)
            nc.vector.memset(SCB[:], 0.0)
            nc.vector.memset(JUNK[:], 0.0)
            nc.vector.memset(SHK[:], 0.0)
            nc.vector.memset(SHP[:], 0.0)
            nc.vector.memset(SH2K[:], 0.0)
            nc.vector.memset(SH2P[:], 0.0)

            # helper views -------------------------------------------------
            def kview(t):
                return t[:].rearrange("p (f a) -> p f a", a=NARR)

            def cview(t, h):
                # bf16 coord view: h=1 -> c0 (high half), h=0 -> c1 (low)
                v = t[:].bitcast(bf16).rearrange(
                    "p (f a h) -> p f a h", a=NARR, h=2)
                return v[:, :, :, h]

            def reductions(kt, pt, final=False):
                """d, and per-batch accumulators into ACC."""
                kv = kview(kt)
                bkv = kview(BK if kt is AK else AK)
                c0 = cview(pt, 1)
                c1 = cview(pt, 0)
                bscr = (BP_ if pt is AP_ else AP_)[:].bitcast(f32).rearrange(
                    "p (f a) -> p f a", a=NARR)
                for b in range(B_PER_CORE):
                    ax, ay = 2 * b, 2 * b + 1
                    D = bkv[:, :, ax]
                    # D = Kx - Ky
                    nc.vector.tensor_tensor(D, kv[:, :, ax], kv[:, :, ay],
                                            Alu.subtract)
                    # sum d^2 (ACT engine, fused square+accum)
                    nc.scalar.activation(bkv[:, :, ay], D, Act.Square,
                                         accum_out=ACC[:, 4 * b:4 * b + 1])
                    if not final:
                        U0 = bscr[:, :, ax]
                        U1 = bscr[:, :, ay]
                        nc.vector.tensor_tensor(U0, c0[:, :, ax], c0[:, :, ay],
                                                Alu.subtract)
                        nc.vector.scalar_tensor_tensor(
                            U0, U0, 1.0, D, Alu.mult, Alu.mult,
                            accum_out=ACC[:, 4 * b + 1:4 * b + 2])
                        nc.vector.tensor_tensor(U1, c1[:, :, ax], c1[:, :, ay],
                                                Alu.subtract)
                        nc.vector.scalar_tensor_tensor(
                            U1, U1, 1.0, D, Alu.mult, Alu.mult,
                            accum_out=ACC[:, 4 * b + 2:4 * b + 3])
                # collect across partitions: PSUM[1,16] = ones^T @ ACC
                nc.tensor.matmul(PSUMT[0:1, :], ONES[:, 0:1], ACC[:, :],
                                 start=True, stop=True)
                nc.scalar.copy(TR[:], PSUMT[0:1, :])

            def adam_and_scalars():
                """TR -> gradient -> adam -> SCOUT (s,alpha,beta per batch) + SCB."""
                r = TR[:].rearrange("o (b q) -> o b q", q=4)
                sd2, su0, su1 = r[:, :, 0], r[:, :, 1], r[:, :, 2]
                tp3 = TP[:].rearrange("o (b c) -> o b c", c=3)
                p0o, p1o, p2o = tp3[:, :, 0], tp3[:, :, 1], tp3[:, :, 2]
                ts4 = TS1[:].rearrange("o (b c) -> o b c", c=3)
                # gp2*0.5 = (sd2 - p0*su0 - p1*su1)/p2
                nc.vector.tensor_tensor(ts4[:, :, 0], su0, p0o, Alu.mult)
                nc.vector.tensor_tensor(ts4[:, :, 1], su1, p1o, Alu.mult)
                nc.vector.tensor_tensor(ts4[:, :, 2], sd2, ts4[:, :, 0],
                                        Alu.subtract)
                nc.vector.tensor_tensor(ts4[:, :, 2], ts4[:, :, 2],
                                        ts4[:, :, 1], Alu.subtract)
                nc.vector.reciprocal(TRC4[:], p2o)
                nc.vector.tensor_tensor(ts4[:, :, 2], ts4[:, :, 2], TRC4[:],
                                        Alu.mult)
                tg3 = TG[:].rearrange("o (b c) -> o b c", c=3)
                nc.vector.tensor_scalar_mul(tg3[:, :, 0], su0, 2.0)
                nc.vector.tensor_scalar_mul(tg3[:, :, 1], su1, 2.0)
                nc.vector.tensor_scalar_mul(tg3[:, :, 2], ts4[:, :, 2], 2.0)
                # dot = sum gp*p per batch; gp_tan = gp - dot*p
                nc.vector.tensor_tensor(TS2[:], TG[:], TP[:], Alu.mult)
                nc.vector.tensor_reduce(
                    TD4[:], TS2[:].rearrange("o (b c) -> o b c", c=3),
                    Axis.X, Alu.add)
                d4b = bcast_inner(TD4[0:1, :], 3)
                nc.vector.tensor_tensor(TS2[:], TP[:], d4b, Alu.mult)
                nc.vector.tensor_tensor(TG[:], TG[:], TS2[:], Alu.subtract)
                # nrm = |u|; gu = -gp_tan/(nrm*32)
                nc.vector.tensor_tensor(TS2[:], TU[:], TU[:], Alu.mult)
                nc.vector.tensor_reduce(
                    TN4[:], TS2[:].rearrange("o (b c) -> o b c", c=3),
                    Axis.X, Alu.add)
                nc.scalar.activation(TN4[:], TN4[:], Act.Sqrt)
                nc.vector.reciprocal(TRC4[:], TN4[:])
                nc.vector.tensor_tensor(TG[:], TG[:], bcast_inner(TRC4[0:1, :], 3),
                                        Alu.mult)
                nc.vector.tensor_scalar_mul(TG[:], TG[:], -1.0 / 32.0)
                # adam moments
                nc.vector.tensor_scalar_mul(TS1[:], TG[:], OneMinusB1)
                nc.vector.scalar_tensor_tensor(TM[:], TM[:], B1f, TS1[:],
                                               Alu.mult, Alu.add)
                nc.vector.tensor_tensor(TS2[:], TG[:], TG[:], Alu.mult)
                nc.vector.tensor_scalar_mul(TS2[:], TS2[:], OneMinusB2)
                nc.vector.scalar_tensor_tensor(TV[:], TV[:], B2f, TS2[:],
                                               Alu.mult, Alu.add)
                # bias correction factors
                nc.vector.tensor_scalar_mul(TBC[0:1, 0:1], TBC[0:1, 0:1], B1f)
                nc.vector.tensor_scalar_mul(TBC[0:1, 1:2], TBC[0:1, 1:2], B2f)
                nc.vector.tensor_scalar(TBCI[:], TBC[:], -1.0, 1.0,
                                        Alu.mult, Alu.add)
                nc.vector.reciprocal(TRC2[:], TBCI[:])
                nc.vector.tensor_tensor(TS1[:], TM[:],
                                        bcast_inner(TRC2[0:1, 0:1], 12),
                                        Alu.mult)
                nc.vector.tensor_tensor(TS2[:], TV[:],
                                        bcast_inner(TRC2[0:1, 1:2], 12),
                                        Alu.mult)
                # u -= lr*mhat/(sqrt(vhat)+eps)
                nc.scalar.activation(TS2[:], TS2[:], Act.Sqrt)
                nc.vector.tensor_scalar_add(TS2[:], TS2[:], EPSf)
                nc.vector.tensor_scalar_mul(TS1[:], TS1[:], LRf)
                nc.vector.reciprocal(TRC12[:], TS2[:])
                nc.vector.tensor_tensor(TS1[:], TS1[:], TRC12[:], Alu.mult)
                nc.vector.tensor_tensor(TU[:], TU[:], TS1[:], Alu.subtract)
                # p_new = u/|u|
                nc.vector.tensor_tensor(TS2[:], TU[:], TU[:], Alu.mult)
                nc.vector.tensor_reduce(
                    TN4[:], TS2[:].rearrange("o (b c) -> o b c", c=3),
                    Axis.X, Alu.add)
                nc.scalar.activation(TN4[:], TN4[:], Act.Sqrt)
                nc.vector.reciprocal(TRC4[:], TN4[:])
                nc.vector.tensor_tensor(TPN[:], TU[:],
                                        bcast_inner(TRC4[0:1, :], 3), Alu.mult)
                # delta and per-batch key-update scalars
                nc.vector.tensor_tensor(TS1[:], TPN[:], TP[:], Alu.subtract)
                dl3 = TS1[:].rearrange("o (b c) -> o b c", c=3)
                sc4 = SCOUT[:].rearrange("o (b q) -> o b q", q=4)
                nc.vector.reciprocal(TRC4[:], p2o)
                nc.vector.tensor_tensor(TD4[:], dl3[:, :, 2], TRC4[:], Alu.mult)
                nc.vector.tensor_scalar_add(sc4[:, :, 0], TD4[:], 1.0)
                nc.vector.tensor_tensor(TN4[:], TD4[:], p0o, Alu.mult)
                nc.vector.tensor_tensor(sc4[:, :, 1], dl3[:, :, 0], TN4[:],
                                        Alu.subtract)
                nc.vector.tensor_tensor(TN4[:], TD4[:], p1o, Alu.mult)
                nc.vector.tensor_tensor(sc4[:, :, 2], dl3[:, :, 1], TN4[:],
                                        Alu.subtract)
                nc.vector.tensor_copy(TP[:], TPN[:])
                # broadcast to all partitions via PE outer product
                nc.tensor.matmul(PSB[:, :], ONESR[0:1, :], SCOUT[:, :],
                                 start=True, stop=True)
                nc.scalar.copy(SCB[:], PSB[:, :])

            def key_update():
                kv = kview(AK)
                c0 = cview(AP_, 1)
                c1 = cview(AP_, 0)
                for b in range(B_PER_CORE):
                    ks = kv[:, :, 2 * b:2 * b + 2]
                    nc.vector.tensor_scalar_mul(ks, ks, SCB[:, 4 * b:4 * b + 1])
                    nc.vector.scalar_tensor_tensor(
                        ks, c0[:, :, 2 * b:2 * b + 2],
                        SCB[:, 4 * b + 1:4 * b + 2], ks, Alu.mult, Alu.add)
                    nc.vector.scalar_tensor_tensor(
                        ks, c1[:, :, 2 * b:2 * b + 2],
                        SCB[:, 4 * b + 2:4 * b + 3], ks, Alu.mult, Alu.add)

            def stage(g, ph, srcK, dstK, srcP, dstP):
                B = FPR // (2 * g)
                for t, s, d in ((0, srcK, dstK), (1, srcP, dstP)):
                    sap = s[:] if t == 0 else s[:].bitcast(f32)
                    dap = d[:] if t == 0 else d[:].bitcast(f32)
                    sv = sap.rearrange("p (b two j a) -> p b two j a",
                                       two=2, j=g, a=NARR)
                    dv = dap.rearrange("p (b two j a) -> p b two j a",
                                       two=2, j=g, a=NARR)
                    if ph == 0:
                        slo, shi = sv[:, :, 0], sv[:, :, 1]
                        dlo, dhi = dv[:, :, 0], dv[:, :, 1]
                        mv = MASK[:].rearrange("p (b j a) -> p b j a",
                                               j=g, a=NARR)
                    else:
                        slo, shi = sv[:, 0:B - 1, 1], sv[:, 1:B, 0]
                        dlo, dhi = dv[:, 0:B - 1, 1], dv[:, 1:B, 0]
                        mv = MASK[:].rearrange("p (b j a) -> p b j a",
                                               j=g, a=NARR)[:, 0:B - 1]
                    if t == 0:
                        nc.vector.tensor_tensor(mv, slo, shi, Alu.is_gt)
                        nc.vector.tensor_tensor(dlo, slo, shi, Alu.min)
                        nc.vector.tensor_tensor(dhi, slo, shi, Alu.max)
                    else:
                        nc.gpsimd.tensor_copy(dlo, slo)
                        nc.scalar.copy(dhi, shi)
                        nc.vector.copy_predicated(dlo, mv, shi)
                        nc.vector.copy_predicated(dhi, mv, slo)
                    if ph == 1:
                        # uncovered row-edge regions: plain copies
                        fv_s = sap.rearrange("p (f a) -> p f a", a=NARR)
                        fv_d = dap.rearrange("p (f a) -> p f a", a=NARR)
                        nc.scalar.copy(fv_d[:, 0:g, :], fv_s[:, 0:g, :])
                        nc.scalar.copy(fv_d[:, FPR - g:FPR, :],
                                       fv_s[:, FPR - g:FPR, :])

            def boundary_event(w, curK, curP):
                """merge-exchange row tails (rows 0..126) vs next-row heads,
                in place on the current buffers, via DMA staging."""
                W = w * NARR
                kf = curK[:].rearrange("p (f a) -> p f a", a=NARR)
                pf = curP[:].bitcast(f32).rearrange("p (f a) -> p f a", a=NARR)
                ktail = kf[0:ROWS - 1, FPR - w:FPR, :]
                ptail = pf[0:ROWS - 1, FPR - w:FPR, :]
                khead = kf[1:ROWS, 0:w, :]
                phead = pf[1:ROWS, 0:w, :]
                shk = SHK[0:ROWS - 1, 0:W]
                shp = SHP[0:ROWS - 1, 0:W]
                sh2k = SH2K[0:ROWS - 1, 0:W]
                sh2p = SH2P[0:ROWS - 1, 0:W]
                m2 = MASK2[0:ROWS - 1, 0:W]
                # stage heads of rows 1.. at partitions 0..
                nc.sync.dma_start(out=shk, in_=khead)
                nc.sync.dma_start(out=shp, in_=phead)
                nc.vector.tensor_tensor(m2, ktail, shk, Alu.is_gt)
                # new head values
                nc.vector.tensor_tensor(sh2k, ktail, shk, Alu.max)
                nc.scalar.copy(sh2p, shp)
                nc.vector.copy_predicated(sh2p, m2, ptail)
                # in-place tail update
                nc.vector.tensor_tensor(ktail, ktail, shk, Alu.min)
                nc.vector.copy_predicated(ptail, m2, shp)
                # write back heads
                nc.sync.dma_start(out=khead, in_=sh2k)
                nc.sync.dma_start(out=phead, in_=sh2p)

            def repair(gaps, bsched):
                bufs = [(AK, AP_), (BK, BP_)]
                cur = 0
                for i, (g, ph) in enumerate(gaps):
                    (sK, sP), (dK, dP) = bufs[cur], bufs[1 - cur]
                    stage(g, ph, sK, dK, sP, dP)
                    cur = 1 - cur
                    if i in bsched:
                        boundary_event(bsched[i], bufs[cur][0], bufs[cur][1])
                assert cur == 0

            # ---------- main loop ----------
            def body(iv):
                reductions(AK, AP_)
                adam_and_scalars()
                key_update()
                repair(GAPS, BSCHED)

            import os as _os
            if _os.environ.get("KERNEL_UNROLL"):
                for _i in range(num_iter):
                    body(_i)
            else:
                with tc.For_i(0, num_iter, 1) as iv:
                    body(iv)

            # ---------- epilogue ----------
            repair(CLEANUP, BSCHED_CLEAN)
            reductions(AK, AP_, final=True)
            nc.sync.dma_start(out=out_d, in_=TR[:])

    nc.compile()
    return nc


_NC_CACHE = {}


def _get_nc(num_iter=NUM_ITER):
    if num_iter not in _NC_CACHE:
        _NC_CACHE[num_iter] = build_nc(num_iter)
    return _NC_CACHE[num_iter]


def _prep_core(xc, yc, pc, num_iter):
    """Host-side prep for one core: returns the in_map."""
    KIN = np.empty((ROWS, FAT), np.float32)
    PIN = np.empty((ROWS, FAT), np.uint32)
    SCIN = np.empty((1, 24), np.float32)
    for b in range(B_PER_CORE):
        u0 = pc[b, 0].astype(np.float32)
        nrm = np.sqrt((u0.astype(np.float32) ** 2).sum(dtype=np.float32))
        p0 = (u0 / nrm).astype(np.float32)
        perm = np.argsort(np.abs(p0), kind="stable")
        xb = xc[b][:, perm]
        yb = yc[b][:, perm]
        p0p = p0[perm]
        u0p = u0[perm]
        SCIN[0, 3 * b:3 * b + 3] = u0p
        SCIN[0, 12 + 3 * b:12 + 3 * b + 3] = p0p
        for cloud, arr in ((0, xb), (1, yb)):
            a = 2 * b + cloud
            proj = (arr @ p0p).astype(np.float32)
            order = np.argsort(proj, kind="stable")
            k = proj[order]
            c0 = arr[order, 0].astype(ml_dtypes.bfloat16)
            c1 = arr[order, 1].astype(ml_dtypes.bfloat16)
            packed = (c0.view(np.uint16).astype(np.uint32) << 16) | \
                c1.view(np.uint16).astype(np.uint32)
            KIN[:, a::NARR] = k.reshape(ROWS, FPR)
            PIN[:, a::NARR] = packed.reshape(ROWS, FPR)
    return {"kin": KIN, "pin": PIN, "scin": SCIN}


def kernel(x, y, proj_init, num_iter=NUM_ITER):
    x = np.asarray(x)
    y = np.asarray(y)
    proj_init = np.asarray(proj_init)
    Btot = x.shape[0]
    assert Btot == NCORES * B_PER_CORE
    nc = _get_nc(num_iter)
    in_maps = []
    for c in range(NCORES):
        sl = slice(c * B_PER_CORE, (c + 1) * B_PER_CORE)
        in_maps.append(_prep_core(x[sl], y[sl], proj_init[sl], num_iter))
    res = run_bass_kernel_spmd(nc, in_maps, core_ids=list(range(NCORES)))
    svals = []
    for c in range(NCORES):
        o = res.results[c]["out"]
        for b in range(B_PER_CORE):
            svals.append(o[0, 4 * b])
    return np.float32(np.mean(np.asarray(svals, np.float64)))


# revision 15
# speedup vs baseline: 86.9497x; 1.0085x over previous
"""Max-SW loss kernel for Trainium2 (8 NeuronCores, data-parallel over batch).

Algorithm (per batch element, 4 per core):
  State: records (K = x@p sort key fp32, packed bf16 coords c0,c1) for both
  point clouds, kept physically sorted; c2 recovered as (K - c0 p0 - c1 p1)/p2
  (host permutes axes so |p2| is maximal).
  Host pre-sorts by the initial projection. Each Adam iteration on-device:
    1. gradient reductions on the position-paired sorted arrays
       g_p = 2 sum_n d_n (X_s[n]-Y_s[n]),  d = Kx - Ky
    2. Adam update of u (3-vector), new direction p, delta = p_new - p_old
    3. incremental key update K <- K*(1+d2/p2) + c0*(d0-d2 p0/p2) + c1*(...)
    4. re-sort repair: 20 decreasing-gap odd-even compare-exchange stages
       (validated numerically: residual disorder stays bounded, final loss
       rel err ~1e-5 vs exact float64 reference)
  Final: cleanup stages + sum d^2 per batch; host averages 32 batches.

Layout: per core 8 arrays (4 batches x {x,y}) interleaved in fat planes
[128, 8192]; rank r = row*1024 + f, fat column index = f*8 + array.
"""
import numpy as np
import ml_dtypes

import concourse.bacc as bacc
import concourse.bass as bass
import concourse.tile as tile
from concourse import mybir
from concourse.bass_utils import run_bass_kernel_spmd

f32 = mybir.dt.float32
u32 = mybir.dt.uint32
u8 = mybir.dt.uint8
bf16 = mybir.dt.bfloat16
Alu = mybir.AluOpType
Act = mybir.ActivationFunctionType
Axis = mybir.AxisListType

NUM_ITER = 50
NCORES = 8
B_PER_CORE = 4
NARR = 8          # arrays per core = 4 batches * (x, y)
ROWS, FPR = 128, 1024   # rank = row*1024 + f
N = ROWS * FPR
FAT = FPR * NARR  # 8192

# exact float32 constants as used by the jax fp32 reference
LRf = float(np.float32(1e-4))
B1f = float(np.float32(0.9))
B2f = float(np.float32(0.999))
OneMinusB1 = float(np.float32(1.0) - np.float32(0.9))
OneMinusB2 = float(np.float32(1.0) - np.float32(0.999))
EPSf = float(np.float32(1e-8))

# (gap, phase) repair schedule, validated in numpy mirror
GAPS = [(64, 0), (64, 1), (32, 0), (32, 1), (16, 0), (16, 1), (8, 0), (8, 1),
        (4, 0), (4, 1), (4, 0), (2, 0), (2, 1), (2, 0), (1, 0), (1, 1),
        (1, 0), (1, 1), (1, 0), (1, 1)]
BSCHED = {1: 64, 7: 32, 13: 8}   # boundary-exchange after stage idx
CLEANUP = [(8, 0), (4, 0), (2, 0), (2, 1), (1, 0), (1, 1)]
BSCHED_CLEAN = {0: 8, 3: 4}
assert len(GAPS) % 2 == 0 and len(CLEANUP) % 2 == 0


def bcast_inner(ap, n):
    """Append a step-0 inner dim of count n to an AP (broadcast)."""
    return bass.AP(tensor=ap.tensor, offset=ap.offset, ap=list(ap.ap) + [[0, n]])


def build_nc(num_iter=NUM_ITER):
    nc = bacc.Bacc("TRN2", target_bir_lowering=False, debug=False,
                   num_devices=NCORES)
    kin = nc.dram_tensor("kin", [ROWS, FAT], f32, kind="ExternalInput").ap()
    pin = nc.dram_tensor("pin", [ROWS, FAT], u32, kind="ExternalInput").ap()
    scin = nc.dram_tensor("scin", [1, 24], f32, kind="ExternalInput").ap()
    out_d = nc.dram_tensor("out", [1, 16], f32, kind="ExternalOutput").ap()

    with tile.TileContext(nc) as tc:
        with (
            tc.tile_pool(name="planes", bufs=1) as planes,
            tc.tile_pool(name="small", bufs=1) as small,
            tc.tile_pool(name="ps", bufs=1, space="PSUM") as psp,
        ):
            AK = planes.tile([ROWS, FAT], f32, tag="AK")
            BK = planes.tile([ROWS, FAT], f32, tag="BK")
            AP_ = planes.tile([ROWS, FAT], u32, tag="AP")
            BP_ = planes.tile([ROWS, FAT], u32, tag="BP")
            MASK = planes.tile([ROWS, 4096], u8, tag="MASK")
            MASK2 = small.tile([ROWS, 512], u8)

            SCB = small.tile([ROWS, 16], f32)
            ACC = small.tile([ROWS, 16], f32)
            ONES = small.tile([ROWS, 1], f32)
            TU = small.tile([1, 12], f32)
            TM = small.tile([1, 12], f32)
            TV = small.tile([1, 12], f32)
            TP = small.tile([1, 12], f32)
            TPN = small.tile([1, 12], f32)
            TG = small.tile([1, 12], f32)
            TS1 = small.tile([1, 12], f32)
            TS2 = small.tile([1, 12], f32)
            TD4 = small.tile([1, 4], f32)
            TN4 = small.tile([1, 4], f32)
            TBC = small.tile([1, 2], f32)
            TBCI = small.tile([1, 2], f32)
            TRC4 = small.tile([1, 4], f32)
            TRC2 = small.tile([1, 2], f32)
            TRC12 = small.tile([1, 12], f32)
            TR = small.tile([1, 16], f32)
            SCOUT = small.tile([1, 16], f32)
            JUNK = small.tile([ROWS, 16], f32)
            ONESR = small.tile([1, ROWS], f32)
            SHK = small.tile([ROWS, 512], f32)
            SHP = small.tile([ROWS, 512], f32)
            SH2K = small.tile([ROWS, 512], f32)
            SH2P = small.tile([ROWS, 512], f32)
            PSUMT = psp.tile([1, 16], f32)
            PSB = psp.tile([ROWS, 16], f32)

            # ---------- prologue ----------
            nc.sync.dma_start(out=AK[:], in_=kin)
            nc.sync.dma_start(out=AP_[:], in_=pin)
            nc.sync.dma_start(out=TU[:], in_=scin[0:1, 0:12])
            nc.sync.dma_start(out=TP[:], in_=scin[0:1, 12:24])
            nc.vector.memset(TM[:], 0.0)
            nc.vector.memset(TV[:], 0.0)
            nc.vector.memset(TBC[:], 1.0)
            nc.vector.memset(ONES[:], 1.0)
            nc.vector.memset(ONESR[:], 1.0)
            nc.vector.memset(BK[:], 0.0)
            nc.vector.memset(BP_[:], 0)
            nc.vector.memset(MASK[:], 0)
            nc.vector.memset(MASK2[:], 0)
            nc.vector.memset(ACC[:], 0.0)
            nc.vector.memset(SCB[:], 0.0)
            nc.vector.memset(JUNK[:], 0.0)
            nc.vector.memset(SHK[:], 0.0)
            nc.vector.memset(SHP[:], 0.0)
            nc.vector.memset(SH2K[:], 0.0)
            nc.vector.memset(SH2P[:], 0.0)

            # helper views -------------------------------------------------
            def kview(t):
                return t[:].rearrange("p (f a) -> p f a", a=NARR)

            def cview(t, h):
                # bf16 coord view: h=1 -> c0 (high half), h=0 -> c1 (low)
                v = t[:].bitcast(bf16).rearrange(
                    "p (f a h) -> p f a h", a=NARR, h=2)
                return v[:, :, :, h]

            def reductions(kt, pt, final=False):
                """d, and per-batch accumulators into ACC."""
                kv = kview(kt)
                bkv = kview(BK if kt is AK else AK)
                c0 = cview(pt, 1)
                c1 = cview(pt, 0)
                bscr = (BP_ if pt is AP_ else AP_)[:].bitcast(f32).rearrange(
                    "p (f a) -> p f a", a=NARR)
                for b in range(B_PER_CORE):
                    ax, ay = 2 * b, 2 * b + 1
                    D = bkv[:, :, ax]
                    # D = Kx - Ky
                    nc.gpsimd.tensor_tensor(D, kv[:, :, ax], kv[:, :, ay],
                                            Alu.subtract)
                    # sum d^2 (ACT engine, fused square+accum)
                    nc.scalar.activation(bkv[:, :, ay], D, Act.Square,
                                         accum_out=ACC[:, 4 * b:4 * b + 1])
                    if not final:
                        U0 = bscr[:, :, ax]
                        U1 = bscr[:, :, ay]
                        nc.gpsimd.tensor_tensor(U0, c0[:, :, ax], c0[:, :, ay],
                                                Alu.subtract)
                        nc.vector.scalar_tensor_tensor(
                            U0, U0, 1.0, D, Alu.mult, Alu.mult,
                            accum_out=ACC[:, 4 * b + 1:4 * b + 2])
                        nc.gpsimd.tensor_tensor(U1, c1[:, :, ax], c1[:, :, ay],
                                                Alu.subtract)
                        nc.vector.scalar_tensor_tensor(
                            U1, U1, 1.0, D, Alu.mult, Alu.mult,
                            accum_out=ACC[:, 4 * b + 2:4 * b + 3])
                # collect across partitions: PSUM[1,16] = ones^T @ ACC
                nc.tensor.matmul(PSUMT[0:1, :], ONES[:, 0:1], ACC[:, :],
                                 start=True, stop=True)
                nc.scalar.copy(TR[:], PSUMT[0:1, :])

            def adam_and_scalars():
                """TR -> gradient -> adam -> SCOUT (s,alpha,beta per batch) + SCB."""
                r = TR[:].rearrange("o (b q) -> o b q", q=4)
                sd2, su0, su1 = r[:, :, 0], r[:, :, 1], r[:, :, 2]
                tp3 = TP[:].rearrange("o (b c) -> o b c", c=3)
                p0o, p1o, p2o = tp3[:, :, 0], tp3[:, :, 1], tp3[:, :, 2]
                ts4 = TS1[:].rearrange("o (b c) -> o b c", c=3)
                # gp2*0.5 = (sd2 - p0*su0 - p1*su1)/p2
                nc.vector.tensor_tensor(ts4[:, :, 0], su0, p0o, Alu.mult)
                nc.vector.tensor_tensor(ts4[:, :, 1], su1, p1o, Alu.mult)
                nc.vector.tensor_tensor(ts4[:, :, 2], sd2, ts4[:, :, 0],
                                        Alu.subtract)
                nc.vector.tensor_tensor(ts4[:, :, 2], ts4[:, :, 2],
                                        ts4[:, :, 1], Alu.subtract)
                nc.vector.reciprocal(TRC4[:], p2o)
                nc.vector.tensor_tensor(ts4[:, :, 2], ts4[:, :, 2], TRC4[:],
                                        Alu.mult)
                tg3 = TG[:].rearrange("o (b c) -> o b c", c=3)
                nc.vector.tensor_scalar_mul(tg3[:, :, 0], su0, 2.0)
                nc.vector.tensor_scalar_mul(tg3[:, :, 1], su1, 2.0)
                nc.vector.tensor_scalar_mul(tg3[:, :, 2], ts4[:, :, 2], 2.0)
                # dot = sum gp*p per batch; gp_tan = gp - dot*p
                nc.vector.tensor_tensor(TS2[:], TG[:], TP[:], Alu.mult)
                nc.vector.tensor_reduce(
                    TD4[:], TS2[:].rearrange("o (b c) -> o b c", c=3),
                    Axis.X, Alu.add)
                d4b = bcast_inner(TD4[0:1, :], 3)
                nc.vector.tensor_tensor(TS2[:], TP[:], d4b, Alu.mult)
                nc.vector.tensor_tensor(TG[:], TG[:], TS2[:], Alu.subtract)
                # nrm = |u|; gu = -gp_tan/(nrm*32)
                nc.vector.tensor_tensor(TS2[:], TU[:], TU[:], Alu.mult)
                nc.vector.tensor_reduce(
                    TN4[:], TS2[:].rearrange("o (b c) -> o b c", c=3),
                    Axis.X, Alu.add)
                nc.scalar.activation(TN4[:], TN4[:], Act.Sqrt)
                nc.vector.reciprocal(TRC4[:], TN4[:])
                nc.vector.tensor_tensor(TG[:], TG[:], bcast_inner(TRC4[0:1, :], 3),
                                        Alu.mult)
                nc.vector.tensor_scalar_mul(TG[:], TG[:], -1.0 / 32.0)
                # adam moments
                nc.vector.tensor_scalar_mul(TS1[:], TG[:], OneMinusB1)
                nc.vector.scalar_tensor_tensor(TM[:], TM[:], B1f, TS1[:],
                                               Alu.mult, Alu.add)
                nc.vector.tensor_tensor(TS2[:], TG[:], TG[:], Alu.mult)
                nc.vector.tensor_scalar_mul(TS2[:], TS2[:], OneMinusB2)
                nc.vector.scalar_tensor_tensor(TV[:], TV[:], B2f, TS2[:],
                                               Alu.mult, Alu.add)
                # bias correction factors
                nc.vector.tensor_scalar_mul(TBC[0:1, 0:1], TBC[0:1, 0:1], B1f)
                nc.vector.tensor_scalar_mul(TBC[0:1, 1:2], TBC[0:1, 1:2], B2f)
                nc.vector.tensor_scalar(TBCI[:], TBC[:], -1.0, 1.0,
                                        Alu.mult, Alu.add)
                nc.vector.reciprocal(TRC2[:], TBCI[:])
                nc.vector.tensor_tensor(TS1[:], TM[:],
                                        bcast_inner(TRC2[0:1, 0:1], 12),
                                        Alu.mult)
                nc.vector.tensor_tensor(TS2[:], TV[:],
                                        bcast_inner(TRC2[0:1, 1:2], 12),
                                        Alu.mult)
                # u -= lr*mhat/(sqrt(vhat)+eps)
                nc.scalar.activation(TS2[:], TS2[:], Act.Sqrt)
                nc.vector.tensor_scalar_add(TS2[:], TS2[:], EPSf)
                nc.vector.tensor_scalar_mul(TS1[:], TS1[:], LRf)
                nc.vector.reciprocal(TRC12[:], TS2[:])
                nc.vector.tensor_tensor(TS1[:], TS1[:], TRC12[:], Alu.mult)
                nc.vector.tensor_tensor(TU[:], TU[:], TS1[:], Alu.subtract)
                # p_new = u/|u|
                nc.vector.tensor_tensor(TS2[:], TU[:], TU[:], Alu.mult)
                nc.vector.tensor_reduce(
                    TN4[:], TS2[:].rearrange("o (b c) -> o b c", c=3),
                    Axis.X, Alu.add)
                nc.scalar.activation(TN4[:], TN4[:], Act.Sqrt)
                nc.vector.reciprocal(TRC4[:], TN4[:])
                nc.vector.tensor_tensor(TPN[:], TU[:],
                                        bcast_inner(TRC4[0:1, :], 3), Alu.mult)
                # delta and per-batch key-update scalars
                nc.vector.tensor_tensor(TS1[:], TPN[:], TP[:], Alu.subtract)
                dl3 = TS1[:].rearrange("o (b c) -> o b c", c=3)
                sc4 = SCOUT[:].rearrange("o (b q) -> o b q", q=4)
                nc.vector.reciprocal(TRC4[:], p2o)
                nc.vector.tensor_tensor(TD4[:], dl3[:, :, 2], TRC4[:], Alu.mult)
                nc.vector.tensor_scalar_add(sc4[:, :, 0], TD4[:], 1.0)
                nc.vector.tensor_tensor(TN4[:], TD4[:], p0o, Alu.mult)
                nc.vector.tensor_tensor(sc4[:, :, 1], dl3[:, :, 0], TN4[:],
                                        Alu.subtract)
                nc.vector.tensor_tensor(TN4[:], TD4[:], p1o, Alu.mult)
                nc.vector.tensor_tensor(sc4[:, :, 2], dl3[:, :, 1], TN4[:],
                                        Alu.subtract)
                nc.vector.tensor_copy(TP[:], TPN[:])
                # broadcast to all partitions via PE outer product
                nc.tensor.matmul(PSB[:, :], ONESR[0:1, :], SCOUT[:, :],
                                 start=True, stop=True)
                nc.scalar.copy(SCB[:], PSB[:, :])

            def key_update():
                kv = kview(AK)
                c0 = cview(AP_, 1)
                c1 = cview(AP_, 0)
                for b in range(B_PER_CORE):
                    ks = kv[:, :, 2 * b:2 * b + 2]
                    nc.vector.tensor_scalar_mul(ks, ks, SCB[:, 4 * b:4 * b + 1])
                    nc.vector.scalar_tensor_tensor(
                        ks, c0[:, :, 2 * b:2 * b + 2],
                        SCB[:, 4 * b + 1:4 * b + 2], ks, Alu.mult, Alu.add)
                    nc.vector.scalar_tensor_tensor(
                        ks, c1[:, :, 2 * b:2 * b + 2],
                        SCB[:, 4 * b + 2:4 * b + 3], ks, Alu.mult, Alu.add)

            def stage(g, ph, srcK, dstK, srcP, dstP):
                B = FPR // (2 * g)
                for t, s, d in ((0, srcK, dstK), (1, srcP, dstP)):
                    sap = s[:] if t == 0 else s[:].bitcast(f32)
                    dap = d[:] if t == 0 else d[:].bitcast(f32)
                    sv = sap.rearrange("p (b two j a) -> p b two j a",
                                       two=2, j=g, a=NARR)
                    dv = dap.rearrange("p (b two j a) -> p b two j a",
                                       two=2, j=g, a=NARR)
                    if ph == 0:
                        slo, shi = sv[:, :, 0], sv[:, :, 1]
                        dlo, dhi = dv[:, :, 0], dv[:, :, 1]
                        mv = MASK[:].rearrange("p (b j a) -> p b j a",
                                               j=g, a=NARR)
                    else:
                        slo, shi = sv[:, 0:B - 1, 1], sv[:, 1:B, 0]
                        dlo, dhi = dv[:, 0:B - 1, 1], dv[:, 1:B, 0]
                        mv = MASK[:].rearrange("p (b j a) -> p b j a",
                                               j=g, a=NARR)[:, 0:B - 1]
                    if t == 0:
                        nc.vector.tensor_tensor(mv, slo, shi, Alu.is_gt)
                        nc.vector.tensor_tensor(dlo, slo, shi, Alu.min)
                        nc.vector.tensor_tensor(dhi, slo, shi, Alu.max)
                    else:
                        nc.gpsimd.tensor_copy(dlo, slo)
                        nc.scalar.copy(dhi, shi)
                        nc.vector.copy_predicated(dlo, mv, shi)
                        nc.vector.copy_predicated(dhi, mv, slo)
                    if ph == 1:
                        # uncovered row-edge regions: plain copies
                        fv_s = sap.rearrange("p (f a) -> p f a", a=NARR)
                        fv_d = dap.rearrange("p (f a) -> p f a", a=NARR)
                        nc.scalar.copy(fv_d[:, 0:g, :], fv_s[:, 0:g, :])
                        nc.scalar.copy(fv_d[:, FPR - g:FPR, :],
                                       fv_s[:, FPR - g:FPR, :])

            def boundary_event(w, curK, curP):
                """merge-exchange row tails (rows 0..126) vs next-row heads,
                in place on the current buffers, via DMA staging."""
                W = w * NARR
                kf = curK[:].rearrange("p (f a) -> p f a", a=NARR)
                pf = curP[:].bitcast(f32).rearrange("p (f a) -> p f a", a=NARR)
                ktail = kf[0:ROWS - 1, FPR - w:FPR, :]
                ptail = pf[0:ROWS - 1, FPR - w:FPR, :]
                khead = kf[1:ROWS, 0:w, :]
                phead = pf[1:ROWS, 0:w, :]
                shk = SHK[0:ROWS - 1, 0:W]
                shp = SHP[0:ROWS - 1, 0:W]
                sh2k = SH2K[0:ROWS - 1, 0:W]
                sh2p = SH2P[0:ROWS - 1, 0:W]
                m2 = MASK2[0:ROWS - 1, 0:W]
                # stage heads of rows 1.. at partitions 0..
                nc.sync.dma_start(out=shk, in_=khead)
                nc.sync.dma_start(out=shp, in_=phead)
                nc.vector.tensor_tensor(m2, ktail, shk, Alu.is_gt)
                # new head values
                nc.vector.tensor_tensor(sh2k, ktail, shk, Alu.max)
                nc.scalar.copy(sh2p, shp)
                nc.vector.copy_predicated(sh2p, m2, ptail)
                # in-place tail update
                nc.vector.tensor_tensor(ktail, ktail, shk, Alu.min)
                nc.vector.copy_predicated(ptail, m2, shp)
                # write back heads
                nc.sync.dma_start(out=khead, in_=sh2k)
                nc.sync.dma_start(out=phead, in_=sh2p)

            def repair(gaps, bsched):
                bufs = [(AK, AP_), (BK, BP_)]
                cur = 0
                for i, (g, ph) in enumerate(gaps):
                    (sK, sP), (dK, dP) = bufs[cur], bufs[1 - cur]
                    stage(g, ph, sK, dK, sP, dP)
                    cur = 1 - cur
                    if i in bsched:
                        boundary_event(bsched[i], bufs[cur][0], bufs[cur][1])
                assert cur == 0

            # ---------- main loop ----------
            def body(iv):
                reductions(AK, AP_)
                adam_and_scalars()
                key_update()
                repair(GAPS, BSCHED)

            import os as _os
            if _os.environ.get("KERNEL_UNROLL"):
                for _i in range(num_iter):
                    body(_i)
            else:
                with tc.For_i(0, num_iter, 1) as iv:
                    body(iv)

            # ---------- epilogue ----------
            repair(CLEANUP, BSCHED_CLEAN)
            reductions(AK, AP_, final=True)
            nc.sync.dma_start(out=out_d, in_=TR[:])

    nc.compile()
    return nc


_NC_CACHE = {}


def _get_nc(num_iter=NUM_ITER):
    if num_iter not in _NC_CACHE:
        _NC_CACHE[num_iter] = build_nc(num_iter)
    return _NC_CACHE[num_iter]


def _prep_core(xc, yc, pc, num_iter):
    """Host-side prep for one core: returns the in_map."""
    KIN = np.empty((ROWS, FAT), np.float32)
    PIN = np.empty((ROWS, FAT), np.uint32)
    SCIN = np.empty((1, 24), np.float32)
    for b in range(B_PER_CORE):
        u0 = pc[b, 0].astype(np.float32)
        nrm = np.sqrt((u0.astype(np.float32) ** 2).sum(dtype=np.float32))
        p0 = (u0 / nrm).astype(np.float32)
        perm = np.argsort(np.abs(p0), kind="stable")
        xb = xc[b][:, perm]
        yb = yc[b][:, perm]
        p0p = p0[perm]
        u0p = u0[perm]
        SCIN[0, 3 * b:3 * b + 3] = u0p
        SCIN[0, 12 + 3 * b:12 + 3 * b + 3] = p0p
        for cloud, arr in ((0, xb), (1, yb)):
            a = 2 * b + cloud
            proj = (arr @ p0p).astype(np.float32)
            order = np.argsort(proj, kind="stable")
            k = proj[order]
            c0 = arr[order, 0].astype(ml_dtypes.bfloat16)
            c1 = arr[order, 1].astype(ml_dtypes.bfloat16)
            packed = (c0.view(np.uint16).astype(np.uint32) << 16) | \
                c1.view(np.uint16).astype(np.uint32)
            KIN[:, a::NARR] = k.reshape(ROWS, FPR)
            PIN[:, a::NARR] = packed.reshape(ROWS, FPR)
    return {"kin": KIN, "pin": PIN, "scin": SCIN}


def kernel(x, y, proj_init, num_iter=NUM_ITER):
    x = np.asarray(x)
    y = np.asarray(y)
    proj_init = np.asarray(proj_init)
    Btot = x.shape[0]
    assert Btot == NCORES * B_PER_CORE
    nc = _get_nc(num_iter)
    in_maps = []
    for c in range(NCORES):
        sl = slice(c * B_PER_CORE, (c + 1) * B_PER_CORE)
        in_maps.append(_prep_core(x[sl], y[sl], proj_init[sl], num_iter))
    res = run_bass_kernel_spmd(nc, in_maps, core_ids=list(range(NCORES)))
    svals = []
    for c in range(NCORES):
        o = res.results[c]["out"]
        for b in range(B_PER_CORE):
            svals.append(o[0, 4 * b])
    return np.float32(np.mean(np.asarray(svals, np.float64)))


# revision 16
# speedup vs baseline: 95.7626x; 1.1014x over previous
"""Max-SW loss kernel for Trainium2 (8 NeuronCores, data-parallel over batch).

Algorithm (per batch element, 4 per core):
  State: records (K = x@p sort key fp32, packed bf16 coords c0,c1) for both
  point clouds, kept physically sorted; c2 recovered as (K - c0 p0 - c1 p1)/p2
  (host permutes axes so |p2| is maximal).
  Host pre-sorts by the initial projection. Each Adam iteration on-device:
    1. gradient reductions on the position-paired sorted arrays
       g_p = 2 sum_n d_n (X_s[n]-Y_s[n]),  d = Kx - Ky
    2. Adam update of u (3-vector), new direction p, delta = p_new - p_old
    3. incremental key update K <- K*(1+d2/p2) + c0*(d0-d2 p0/p2) + c1*(...)
    4. re-sort repair: 20 decreasing-gap odd-even compare-exchange stages
       (validated numerically: residual disorder stays bounded, final loss
       rel err ~1e-5 vs exact float64 reference)
  Final: cleanup stages + sum d^2 per batch; host averages 32 batches.

Layout: per core 8 arrays (4 batches x {x,y}) interleaved in fat planes
[128, 8192]; rank r = row*1024 + f, fat column index = f*8 + array.
"""
import numpy as np
import ml_dtypes

import concourse.bacc as bacc
import concourse.bass as bass
import concourse.tile as tile
from concourse import mybir
from concourse.bass_utils import run_bass_kernel_spmd

f32 = mybir.dt.float32
u32 = mybir.dt.uint32
u8 = mybir.dt.uint8
bf16 = mybir.dt.bfloat16
Alu = mybir.AluOpType
Act = mybir.ActivationFunctionType
Axis = mybir.AxisListType

NUM_ITER = 50
NCORES = 8
B_PER_CORE = 4
NARR = 8          # arrays per core = 4 batches * (x, y)
ROWS, FPR = 128, 1024   # rank = row*1024 + f
N = ROWS * FPR
FAT = FPR * NARR  # 8192

# exact float32 constants as used by the jax fp32 reference
LRf = float(np.float32(1e-4))
B1f = float(np.float32(0.9))
B2f = float(np.float32(0.999))
OneMinusB1 = float(np.float32(1.0) - np.float32(0.9))
OneMinusB2 = float(np.float32(1.0) - np.float32(0.999))
EPSf = float(np.float32(1e-8))

# (gap, phase) repair schedule, validated in numpy mirror
GAPS = [(64, 0), (64, 1), (32, 0), (32, 1), (16, 0), (16, 1), (8, 0), (8, 1),
        (4, 0), (4, 1), (4, 0), (2, 0), (2, 1), (2, 0), (1, 0), (1, 1),
        (1, 0), (1, 1), (1, 0), (1, 1)]
BSCHED = {1: 64, 7: 32, 13: 8}   # boundary-exchange after stage idx
CLEANUP = [(8, 0), (4, 0), (2, 0), (2, 1), (1, 0), (1, 1)]
BSCHED_CLEAN = {0: 8, 3: 4}
assert len(GAPS) % 2 == 0 and len(CLEANUP) % 2 == 0


def bcast_inner(ap, n):
    """Append a step-0 inner dim of count n to an AP (broadcast)."""
    return bass.AP(tensor=ap.tensor, offset=ap.offset, ap=list(ap.ap) + [[0, n]])


def build_nc(num_iter=NUM_ITER):
    nc = bacc.Bacc("TRN2", target_bir_lowering=False, debug=False,
                   num_devices=NCORES)
    kin = nc.dram_tensor("kin", [ROWS, FAT], f32, kind="ExternalInput").ap()
    pin = nc.dram_tensor("pin", [ROWS, FAT], u32, kind="ExternalInput").ap()
    scin = nc.dram_tensor("scin", [1, 24], f32, kind="ExternalInput").ap()
    out_d = nc.dram_tensor("out", [1, 16], f32, kind="ExternalOutput").ap()

    with tile.TileContext(nc) as tc:
        with (
            tc.tile_pool(name="planes", bufs=1) as planes,
            tc.tile_pool(name="small", bufs=1) as small,
            tc.tile_pool(name="ps", bufs=1, space="PSUM") as psp,
        ):
            AK = planes.tile([ROWS, FAT], f32, tag="AK")
            BK = planes.tile([ROWS, FAT], f32, tag="BK")
            AP_ = planes.tile([ROWS, FAT], u32, tag="AP")
            BP_ = planes.tile([ROWS, FAT], u32, tag="BP")
            MASK = planes.tile([ROWS, 4096], u8, tag="MASK")
            MASKB = planes.tile([ROWS, 4096], u8, tag="MASKB")
            DSCA = planes.tile([ROWS, 4096], f32, tag="DSCA")
            DSCB = planes.tile([ROWS, 4096], f32, tag="DSCB")
            MASK2 = small.tile([ROWS, 512], u8)

            SCB = small.tile([ROWS, 16], f32)
            ACC = small.tile([ROWS, 16], f32)
            ONES = small.tile([ROWS, 1], f32)
            TU = small.tile([1, 12], f32)
            TM = small.tile([1, 12], f32)
            TV = small.tile([1, 12], f32)
            TP = small.tile([1, 12], f32)
            TPN = small.tile([1, 12], f32)
            TG = small.tile([1, 12], f32)
            TS1 = small.tile([1, 12], f32)
            TS2 = small.tile([1, 12], f32)
            TD4 = small.tile([1, 4], f32)
            TN4 = small.tile([1, 4], f32)
            TBC = small.tile([1, 2], f32)
            TBCI = small.tile([1, 2], f32)
            TRC4 = small.tile([1, 4], f32)
            TRC2 = small.tile([1, 2], f32)
            TRC12 = small.tile([1, 12], f32)
            TR = small.tile([1, 16], f32)
            SCOUT = small.tile([1, 16], f32)
            JUNK = small.tile([ROWS, 16], f32)
            ONESR = small.tile([1, ROWS], f32)
            SHK = small.tile([ROWS, 512], f32)
            SHP = small.tile([ROWS, 512], f32)
            SH2K = small.tile([ROWS, 512], f32)
            SH2P = small.tile([ROWS, 512], f32)
            PSUMT = psp.tile([1, 16], f32)
            PSB = psp.tile([ROWS, 16], f32)

            # ---------- prologue ----------
            nc.sync.dma_start(out=AK[:], in_=kin)
            nc.sync.dma_start(out=AP_[:], in_=pin)
            nc.sync.dma_start(out=TU[:], in_=scin[0:1, 0:12])
            nc.sync.dma_start(out=TP[:], in_=scin[0:1, 12:24])
            nc.vector.memset(TM[:], 0.0)
            nc.vector.memset(TV[:], 0.0)
            nc.vector.memset(TBC[:], 1.0)
            nc.vector.memset(ONES[:], 1.0)
            nc.vector.memset(ONESR[:], 1.0)
            nc.vector.memset(BK[:], 0.0)
            nc.vector.memset(BP_[:], 0)
            nc.vector.memset(MASK[:], 0)
            nc.vector.memset(MASKB[:], 0)
            nc.vector.memset(DSCA[:], 0.0)
            nc.vector.memset(DSCB[:], 0.0)
            nc.vector.memset(MASK2[:], 0)
            nc.vector.memset(ACC[:], 0.0)
            nc.vector.memset(SCB[:], 0.0)
            nc.vector.memset(JUNK[:], 0.0)
            nc.vector.memset(SHK[:], 0.0)
            nc.vector.memset(SHP[:], 0.0)
            nc.vector.memset(SH2K[:], 0.0)
            nc.vector.memset(SH2P[:], 0.0)

            # helper views -------------------------------------------------
            def kview(t):
                return t[:].rearrange("p (f a) -> p f a", a=NARR)

            def cview(t, h):
                # bf16 coord view: h=1 -> c0 (high half), h=0 -> c1 (low)
                v = t[:].bitcast(bf16).rearrange(
                    "p (f a h) -> p f a h", a=NARR, h=2)
                return v[:, :, :, h]

            def reductions(kt, pt, final=False):
                """d, and per-batch accumulators into ACC."""
                kv = kview(kt)
                bkv = kview(BK if kt is AK else AK)
                c0 = cview(pt, 1)
                c1 = cview(pt, 0)
                bscr = (BP_ if pt is AP_ else AP_)[:].bitcast(f32).rearrange(
                    "p (f a) -> p f a", a=NARR)
                for b in range(B_PER_CORE):
                    ax, ay = 2 * b, 2 * b + 1
                    D = bkv[:, :, ax]
                    # D = Kx - Ky
                    nc.gpsimd.tensor_tensor(D, kv[:, :, ax], kv[:, :, ay],
                                            Alu.subtract)
                    # sum d^2 (ACT engine, fused square+accum)
                    nc.scalar.activation(bkv[:, :, ay], D, Act.Square,
                                         accum_out=ACC[:, 4 * b:4 * b + 1])
                    if not final:
                        U0 = bscr[:, :, ax]
                        U1 = bscr[:, :, ay]
                        nc.gpsimd.tensor_tensor(U0, c0[:, :, ax], c0[:, :, ay],
                                                Alu.subtract)
                        nc.vector.scalar_tensor_tensor(
                            U0, U0, 1.0, D, Alu.mult, Alu.mult,
                            accum_out=ACC[:, 4 * b + 1:4 * b + 2])
                        nc.gpsimd.tensor_tensor(U1, c1[:, :, ax], c1[:, :, ay],
                                                Alu.subtract)
                        nc.vector.scalar_tensor_tensor(
                            U1, U1, 1.0, D, Alu.mult, Alu.mult,
                            accum_out=ACC[:, 4 * b + 2:4 * b + 3])
                # collect across partitions: PSUM[1,16] = ones^T @ ACC
                nc.tensor.matmul(PSUMT[0:1, :], ONES[:, 0:1], ACC[:, :],
                                 start=True, stop=True)
                nc.scalar.copy(TR[:], PSUMT[0:1, :])

            def adam_and_scalars():
                """TR -> gradient -> adam -> SCOUT (s,alpha,beta per batch) + SCB."""
                r = TR[:].rearrange("o (b q) -> o b q", q=4)
                sd2, su0, su1 = r[:, :, 0], r[:, :, 1], r[:, :, 2]
                tp3 = TP[:].rearrange("o (b c) -> o b c", c=3)
                p0o, p1o, p2o = tp3[:, :, 0], tp3[:, :, 1], tp3[:, :, 2]
                ts4 = TS1[:].rearrange("o (b c) -> o b c", c=3)
                # gp2*0.5 = (sd2 - p0*su0 - p1*su1)/p2
                nc.vector.tensor_tensor(ts4[:, :, 0], su0, p0o, Alu.mult)
                nc.vector.tensor_tensor(ts4[:, :, 1], su1, p1o, Alu.mult)
                nc.vector.tensor_tensor(ts4[:, :, 2], sd2, ts4[:, :, 0],
                                        Alu.subtract)
                nc.vector.tensor_tensor(ts4[:, :, 2], ts4[:, :, 2],
                                        ts4[:, :, 1], Alu.subtract)
                nc.vector.reciprocal(TRC4[:], p2o)
                nc.vector.tensor_tensor(ts4[:, :, 2], ts4[:, :, 2], TRC4[:],
                                        Alu.mult)
                tg3 = TG[:].rearrange("o (b c) -> o b c", c=3)
                nc.vector.tensor_scalar_mul(tg3[:, :, 0], su0, 2.0)
                nc.vector.tensor_scalar_mul(tg3[:, :, 1], su1, 2.0)
                nc.vector.tensor_scalar_mul(tg3[:, :, 2], ts4[:, :, 2], 2.0)
                # dot = sum gp*p per batch; gp_tan = gp - dot*p
                nc.vector.tensor_tensor(TS2[:], TG[:], TP[:], Alu.mult)
                nc.vector.tensor_reduce(
                    TD4[:], TS2[:].rearrange("o (b c) -> o b c", c=3),
                    Axis.X, Alu.add)
                d4b = bcast_inner(TD4[0:1, :], 3)
                nc.vector.tensor_tensor(TS2[:], TP[:], d4b, Alu.mult)
                nc.vector.tensor_tensor(TG[:], TG[:], TS2[:], Alu.subtract)
                # nrm = |u|; gu = -gp_tan/(nrm*32)
                nc.vector.tensor_tensor(TS2[:], TU[:], TU[:], Alu.mult)
                nc.vector.tensor_reduce(
                    TN4[:], TS2[:].rearrange("o (b c) -> o b c", c=3),
                    Axis.X, Alu.add)
                nc.scalar.activation(TN4[:], TN4[:], Act.Sqrt)
                nc.vector.reciprocal(TRC4[:], TN4[:])
                nc.vector.tensor_tensor(TG[:], TG[:], bcast_inner(TRC4[0:1, :], 3),
                                        Alu.mult)
                nc.vector.tensor_scalar_mul(TG[:], TG[:], -1.0 / 32.0)
                # adam moments
                nc.vector.tensor_scalar_mul(TS1[:], TG[:], OneMinusB1)
                nc.vector.scalar_tensor_tensor(TM[:], TM[:], B1f, TS1[:],
                                               Alu.mult, Alu.add)
                nc.vector.tensor_tensor(TS2[:], TG[:], TG[:], Alu.mult)
                nc.vector.tensor_scalar_mul(TS2[:], TS2[:], OneMinusB2)
                nc.vector.scalar_tensor_tensor(TV[:], TV[:], B2f, TS2[:],
                                               Alu.mult, Alu.add)
                # bias correction factors
                nc.vector.tensor_scalar_mul(TBC[0:1, 0:1], TBC[0:1, 0:1], B1f)
                nc.vector.tensor_scalar_mul(TBC[0:1, 1:2], TBC[0:1, 1:2], B2f)
                nc.vector.tensor_scalar(TBCI[:], TBC[:], -1.0, 1.0,
                                        Alu.mult, Alu.add)
                nc.vector.reciprocal(TRC2[:], TBCI[:])
                nc.vector.tensor_tensor(TS1[:], TM[:],
                                        bcast_inner(TRC2[0:1, 0:1], 12),
                                        Alu.mult)
                nc.vector.tensor_tensor(TS2[:], TV[:],
                                        bcast_inner(TRC2[0:1, 1:2], 12),
                                        Alu.mult)
                # u -= lr*mhat/(sqrt(vhat)+eps)
                nc.scalar.activation(TS2[:], TS2[:], Act.Sqrt)
                nc.vector.tensor_scalar_add(TS2[:], TS2[:], EPSf)
                nc.vector.tensor_scalar_mul(TS1[:], TS1[:], LRf)
                nc.vector.reciprocal(TRC12[:], TS2[:])
                nc.vector.tensor_tensor(TS1[:], TS1[:], TRC12[:], Alu.mult)
                nc.vector.tensor_tensor(TU[:], TU[:], TS1[:], Alu.subtract)
                # p_new = u/|u|
                nc.vector.tensor_tensor(TS2[:], TU[:], TU[:], Alu.mult)
                nc.vector.tensor_reduce(
                    TN4[:], TS2[:].rearrange("o (b c) -> o b c", c=3),
                    Axis.X, Alu.add)
                nc.scalar.activation(TN4[:], TN4[:], Act.Sqrt)
                nc.vector.reciprocal(TRC4[:], TN4[:])
                nc.vector.tensor_tensor(TPN[:], TU[:],
                                        bcast_inner(TRC4[0:1, :], 3), Alu.mult)
                # delta and per-batch key-update scalars
                nc.vector.tensor_tensor(TS1[:], TPN[:], TP[:], Alu.subtract)
                dl3 = TS1[:].rearrange("o (b c) -> o b c", c=3)
                sc4 = SCOUT[:].rearrange("o (b q) -> o b q", q=4)
                nc.vector.reciprocal(TRC4[:], p2o)
                nc.vector.tensor_tensor(TD4[:], dl3[:, :, 2], TRC4[:], Alu.mult)
                nc.vector.tensor_scalar_add(sc4[:, :, 0], TD4[:], 1.0)
                nc.vector.tensor_tensor(TN4[:], TD4[:], p0o, Alu.mult)
                nc.vector.tensor_tensor(sc4[:, :, 1], dl3[:, :, 0], TN4[:],
                                        Alu.subtract)
                nc.vector.tensor_tensor(TN4[:], TD4[:], p1o, Alu.mult)
                nc.vector.tensor_tensor(sc4[:, :, 2], dl3[:, :, 1], TN4[:],
                                        Alu.subtract)
                nc.vector.tensor_copy(TP[:], TPN[:])
                # broadcast to all partitions via PE outer product
                nc.tensor.matmul(PSB[:, :], ONESR[0:1, :], SCOUT[:, :],
                                 start=True, stop=True)
                nc.scalar.copy(SCB[:], PSB[:, :])

            def key_update():
                kv = kview(AK)
                c0 = cview(AP_, 1)
                c1 = cview(AP_, 0)
                for b in range(B_PER_CORE):
                    ks = kv[:, :, 2 * b:2 * b + 2]
                    nc.vector.tensor_scalar_mul(ks, ks, SCB[:, 4 * b:4 * b + 1])
                    nc.vector.scalar_tensor_tensor(
                        ks, c0[:, :, 2 * b:2 * b + 2],
                        SCB[:, 4 * b + 1:4 * b + 2], ks, Alu.mult, Alu.add)
                    nc.vector.scalar_tensor_tensor(
                        ks, c1[:, :, 2 * b:2 * b + 2],
                        SCB[:, 4 * b + 2:4 * b + 3], ks, Alu.mult, Alu.add)

            def stage(g, ph, srcK, dstK, srcP, dstP, par=0):
                B = FPR // (2 * g)
                mbuf = MASK if par == 0 else MASKB
                dbuf = DSCA if par == 0 else DSCB
                for t, s, d in ((0, srcK, dstK), (1, srcP, dstP)):
                    sap = s[:] if t == 0 else s[:].bitcast(f32)
                    dap = d[:] if t == 0 else d[:].bitcast(f32)
                    sv = sap.rearrange("p (b two j a) -> p b two j a",
                                       two=2, j=g, a=NARR)
                    dv = dap.rearrange("p (b two j a) -> p b two j a",
                                       two=2, j=g, a=NARR)
                    if ph == 0:
                        slo, shi = sv[:, :, 0], sv[:, :, 1]
                        dlo, dhi = dv[:, :, 0], dv[:, :, 1]
                        mv = mbuf[:].rearrange("p (b j a) -> p b j a",
                                               j=g, a=NARR)
                        dsv = dbuf[:].rearrange("p (b j a) -> p b j a",
                                                j=g, a=NARR)
                    else:
                        slo, shi = sv[:, 0:B - 1, 1], sv[:, 1:B, 0]
                        dlo, dhi = dv[:, 0:B - 1, 1], dv[:, 1:B, 0]
                        mv = mbuf[:].rearrange("p (b j a) -> p b j a",
                                               j=g, a=NARR)[:, 0:B - 1]
                        dsv = dbuf[:].rearrange("p (b j a) -> p b j a",
                                                j=g, a=NARR)[:, 0:B - 1]
                    if t == 0:
                        # mask = Sign(Relu(lo-hi)) on GPSIMD+ACT, freeing DVE
                        nc.gpsimd.tensor_tensor(dsv, slo, shi, Alu.subtract)
                        nc.scalar.activation(dsv, dsv, Act.Relu)
                        nc.scalar.activation(mv, dsv, Act.Sign)
                        nc.vector.tensor_tensor(dlo, slo, shi, Alu.min)
                        nc.vector.tensor_tensor(dhi, slo, shi, Alu.max)
                    else:
                        nc.gpsimd.tensor_copy(dlo, slo)
                        nc.scalar.copy(dhi, shi)
                        nc.vector.copy_predicated(dlo, mv, shi)
                        nc.vector.copy_predicated(dhi, mv, slo)
                    if ph == 1:
                        # uncovered row-edge regions: plain copies
                        fv_s = sap.rearrange("p (f a) -> p f a", a=NARR)
                        fv_d = dap.rearrange("p (f a) -> p f a", a=NARR)
                        nc.scalar.copy(fv_d[:, 0:g, :], fv_s[:, 0:g, :])
                        nc.scalar.copy(fv_d[:, FPR - g:FPR, :],
                                       fv_s[:, FPR - g:FPR, :])

            def boundary_event(w, curK, curP):
                """merge-exchange row tails (rows 0..126) vs next-row heads,
                in place on the current buffers, via DMA staging."""
                W = w * NARR
                kf = curK[:].rearrange("p (f a) -> p f a", a=NARR)
                pf = curP[:].bitcast(f32).rearrange("p (f a) -> p f a", a=NARR)
                ktail = kf[0:ROWS - 1, FPR - w:FPR, :]
                ptail = pf[0:ROWS - 1, FPR - w:FPR, :]
                khead = kf[1:ROWS, 0:w, :]
                phead = pf[1:ROWS, 0:w, :]
                shk = SHK[0:ROWS - 1, 0:W]
                shp = SHP[0:ROWS - 1, 0:W]
                sh2k = SH2K[0:ROWS - 1, 0:W]
                sh2p = SH2P[0:ROWS - 1, 0:W]
                m2 = MASK2[0:ROWS - 1, 0:W]
                # stage heads of rows 1.. at partitions 0..
                nc.sync.dma_start(out=shk, in_=khead)
                nc.sync.dma_start(out=shp, in_=phead)
                nc.vector.tensor_tensor(m2, ktail, shk, Alu.is_gt)
                # new head values
                nc.vector.tensor_tensor(sh2k, ktail, shk, Alu.max)
                nc.scalar.copy(sh2p, shp)
                nc.vector.copy_predicated(sh2p, m2, ptail)
                # in-place tail update
                nc.vector.tensor_tensor(ktail, ktail, shk, Alu.min)
                nc.vector.copy_predicated(ptail, m2, shp)
                # write back heads
                nc.sync.dma_start(out=khead, in_=sh2k)
                nc.sync.dma_start(out=phead, in_=sh2p)

            def repair(gaps, bsched):
                bufs = [(AK, AP_), (BK, BP_)]
                cur = 0
                for i, (g, ph) in enumerate(gaps):
                    (sK, sP), (dK, dP) = bufs[cur], bufs[1 - cur]
                    stage(g, ph, sK, dK, sP, dP, par=i % 2)
                    cur = 1 - cur
                    if i in bsched:
                        boundary_event(bsched[i], bufs[cur][0], bufs[cur][1])
                assert cur == 0

            # ---------- main loop ----------
            def body(iv):
                reductions(AK, AP_)
                adam_and_scalars()
                key_update()
                repair(GAPS, BSCHED)

            import os as _os
            if _os.environ.get("KERNEL_UNROLL"):
                for _i in range(num_iter):
                    body(_i)
            else:
                with tc.For_i(0, num_iter, 1) as iv:
                    body(iv)

            # ---------- epilogue ----------
            repair(CLEANUP, BSCHED_CLEAN)
            reductions(AK, AP_, final=True)
            nc.sync.dma_start(out=out_d, in_=TR[:])

    nc.compile()
    return nc


_NC_CACHE = {}


def _get_nc(num_iter=NUM_ITER):
    if num_iter not in _NC_CACHE:
        _NC_CACHE[num_iter] = build_nc(num_iter)
    return _NC_CACHE[num_iter]


def _prep_core(xc, yc, pc, num_iter):
    """Host-side prep for one core: returns the in_map."""
    KIN = np.empty((ROWS, FAT), np.float32)
    PIN = np.empty((ROWS, FAT), np.uint32)
    SCIN = np.empty((1, 24), np.float32)
    for b in range(B_PER_CORE):
        u0 = pc[b, 0].astype(np.float32)
        nrm = np.sqrt((u0.astype(np.float32) ** 2).sum(dtype=np.float32))
        p0 = (u0 / nrm).astype(np.float32)
        perm = np.argsort(np.abs(p0), kind="stable")
        xb = xc[b][:, perm]
        yb = yc[b][:, perm]
        p0p = p0[perm]
        u0p = u0[perm]
        SCIN[0, 3 * b:3 * b + 3] = u0p
        SCIN[0, 12 + 3 * b:12 + 3 * b + 3] = p0p
        for cloud, arr in ((0, xb), (1, yb)):
            a = 2 * b + cloud
            proj = (arr @ p0p).astype(np.float32)
            order = np.argsort(proj, kind="stable")
            k = proj[order]
            c0 = arr[order, 0].astype(ml_dtypes.bfloat16)
            c1 = arr[order, 1].astype(ml_dtypes.bfloat16)
            packed = (c0.view(np.uint16).astype(np.uint32) << 16) | \
                c1.view(np.uint16).astype(np.uint32)
            KIN[:, a::NARR] = k.reshape(ROWS, FPR)
            PIN[:, a::NARR] = packed.reshape(ROWS, FPR)
    return {"kin": KIN, "pin": PIN, "scin": SCIN}


def kernel(x, y, proj_init, num_iter=NUM_ITER):
    x = np.asarray(x)
    y = np.asarray(y)
    proj_init = np.asarray(proj_init)
    Btot = x.shape[0]
    assert Btot == NCORES * B_PER_CORE
    nc = _get_nc(num_iter)
    in_maps = []
    for c in range(NCORES):
        sl = slice(c * B_PER_CORE, (c + 1) * B_PER_CORE)
        in_maps.append(_prep_core(x[sl], y[sl], proj_init[sl], num_iter))
    res = run_bass_kernel_spmd(nc, in_maps, core_ids=list(range(NCORES)))
    svals = []
    for c in range(NCORES):
        o = res.results[c]["out"]
        for b in range(B_PER_CORE):
            svals.append(o[0, 4 * b])
    return np.float32(np.mean(np.asarray(svals, np.float64)))
